# revision 28
# baseline (speedup 1.0000x reference)
"""Trainium2 Bass kernel for an 8-batch single-head attention block.

Reference computation (per batch b of 8, S=2048 seq, D=A=768):
    Q = relu(X Wq + bq); K = relu(X Wk + bk); V = relu(X Wv + bv)
    P = softmax(Q K^T)          (no 1/sqrt(d) scale)
    X1 = LN(X + P V)
    X2 = LN(X1 + X1 Wd + bd)    (LN affines are identity in this problem)

Sharding: data-parallel - batch b -> NeuronCore b (8 cores, no collectives).

Optimization history (baseline v2 359.7us -> ~331us), keyed to traces:
  * Single ACT table set: walrus assigns each activation to the first
    act_func_set containing it, so exp (scores) + sqrt (LN) thrashed 20
    ACT_TABLE_LOADs x 1.28us with ~0.6-1us PE stalls behind each.  The
    compile masks the table list down to natural_log_exp_and_others
    (relu/exp/ln/square/identity) and rstd = exp(-0.5*ln(var+eps)).
  * Startup DMA: the two HWDGE queues (sync/scalar) are FIFO and share
    ~430 GB/s of HBM; a descriptor's semaphore fires only when all 16
    DMA engines finish its slices, so any non-critical bytes in flight
    delay the critical ones.  Wave 1 carries exactly what the first K
    e-groups consume (xt chunk 0 in thirds + wk e-blocks, interleaved
    across both queues in need order); later waves are released by
    phase-B progress via 1-column "gate copies" into the DMA target
    (WAW makes the DMA wait; it then overwrites the gate column).
  * HAM warmup: 10 dummy N=512 matmuls bridge the DMA wait so the
    first real matmuls run at 2.4 GHz, and dummy identity matmuls in
    the last chunk's tails keep the clock warm through the endgame
    (idle gaps > ~3.4us re-throttle the PE to 1.2 GHz).
  * Center-early tail: transpose the CENTERED residual
    rb = bf16(r - mu) (needs only the row-sum accumulators, not
    var/rstd).  Centering makes X1 @ Wd == rstd * ((r-mu) @ Wd)
    exactly, so the dense proj starts ~2.4us earlier and the LN1
    apply + nmr disappear.  y = (pp * rstd1) + (rb * rstd1 + bd).
    rb runs on the scalar engine (Identity + per-partition bias) so
    the transpose path doesn't queue behind DVE backlog.
  * Scalar-engine offloads: activation(Square, accum_out) replaces the
    DVE sum-of-squares pass; the LN2 apply is Identity(scale=rstd2,
    bias=nmr2).  (Pool/gpsimd cannot run TensorScalarPtr - walrus
    rejects it at codegen.)
  * Endgame: the last rows' LN2 squares are split in halves so the
    first overlaps its producer, the final apply is split ACT/DVE,
    and the two final stores issue on different queues.
  * Output stores bf16 (host upcasts; ~2e-3 of the 2e-2 rel budget).
"""

from contextlib import ExitStack

import numpy as np
import ml_dtypes

import concourse.bass as bass
import concourse.mybir as mybir
import concourse.tile as tile
from concourse import bacc
from concourse.bass_utils import run_bass_kernel_spmd
from concourse.masks import make_identity

S, D = 2048, 768
N_CORES = 8
SB, DB = S // 128, D // 128  # 16 s-blocks, 6 d-blocks
SCH = 512   # phase-B s-chunk width
QCH = 512   # phase-C q-chunk width
NCH = S // SCH
F32 = mybir.dt.float32
BF16 = mybir.dt.bfloat16
AF = mybir.ActivationFunctionType
ALU = mybir.AluOpType
EPS = 1e-5
BF16NP = ml_dtypes.bfloat16

ACT_SET = "natural_log_exp_and_others"  # relu+exp+ln+square+identity


def _split_matmul_waits(nc):
    """Walrus allows only one semaphore wait on self-loading (fp32/fp32r/
    transpose) Matmult instructions; move extra waits onto a preceding
    InstEventSemaphore (which may carry two waits each)."""
    for bb in nc.main_func.blocks:
        new_insts = []
        for inst in bb.instructions:
            if isinstance(inst, mybir.InstMatmult) and inst.sync_info is not None \
                    and len(inst.sync_info.on_wait) > 1:
                waits = list(inst.sync_info.on_wait)
                extra, keep = waits[:-1], waits[-1:]
                while extra:
                    chunk, extra = extra[:2], extra[2:]
                    ev = mybir.InstEventSemaphore(
                        name=nc.get_next_instruction_name(), ins=[], outs=[])
                    ev.engine = inst.engine
                    ev.sync_info = mybir.SyncInfo(on_wait=chunk, on_update=[])
                    nc.register_instruction(ev)
                    new_insts.append(ev)
                inst.sync_info = mybir.SyncInfo(
                    on_wait=keep, on_update=list(inst.sync_info.on_update))
            new_insts.append(inst)
        bb.instructions[:] = new_insts


def _build():
    nc = bacc.Bacc("TRN2", target_bir_lowering=False, debug=False,
                   enable_asserts=False, num_devices=N_CORES)

    xt_d = nc.dram_tensor("xt", [NCH, 128, DB * SCH], BF16,
                          kind="ExternalInput").ap()
    xr_d = nc.dram_tensor("xr", [NCH, 128, (QCH // 128) * D], F32,
                          kind="ExternalInput").ap()
    wkem_d = nc.dram_tensor("wkem", [128, DB * DB * 128], BF16,
                            kind="ExternalInput").ap()
    wqem_d = nc.dram_tensor("wqem", [128, DB * DB * 128], BF16,
                            kind="ExternalInput").ap()
    wv_d = nc.dram_tensor("wv", [128, DB * (D + 2)], BF16,
                          kind="ExternalInput").ap()
    wd_d = nc.dram_tensor("wd", [128, DB * D], BF16, kind="ExternalInput").ap()
    bqk_d = nc.dram_tensor("bqk", [128, 2 * DB], F32, kind="ExternalInput").ap()
    bv_d = nc.dram_tensor("bv", [128, D + 2], F32, kind="ExternalInput").ap()
    bd_d = nc.dram_tensor("bd", [128, D], F32, kind="ExternalInput").ap()
    out_d = nc.dram_tensor("out", [S, D], BF16, kind="ExternalOutput").ap()

    with tile.TileContext(nc) as tc, ExitStack() as ctx:
        consts = ctx.enter_context(tc.tile_pool(name="consts", bufs=1))
        pers = ctx.enter_context(tc.tile_pool(name="pers", bufs=1))
        wdp = ctx.enter_context(tc.tile_pool(name="wdp", bufs=1))

        # bqk first on the scalar queue (tiny - K-relu biases needed early;
        # keeps the sync queue's first descriptor = the critical xt chunk)
        bqk_sb = consts.tile([128, 2 * DB], F32, tag="bqk", name="bqk")
        nc.scalar.dma_start(bqk_sb[:], bqk_d[:])
        bq_sb = [bqk_sb[:, e:e + 1] for e in range(DB)]
        bk_sb = [bqk_sb[:, DB + e:DB + e + 1] for e in range(DB)]

        # persistent bf16 operand tiles
        kt = {}
        qt = {}
        v_sb = []
        for k in range(SB):
            v_sb.append(pers.tile([128, D + 2], BF16, tag=f"v{k}", name=f"v{k}"))

        # ---------------- Phase B: K^T, Q^T, V (all resident, bf16)
        with tc.tile_pool(name="wqkv", bufs=1) as wpool, \
             tc.tile_pool(name="xtp", bufs=1) as xtp, \
             tc.tile_pool(name="bvb", bufs=2) as bvb, \
             tc.tile_pool(name="bpm", bufs=4, space="PSUM") as bpm:
            # HAM warmup: ~3.4us of dummy matmuls on a DVE-memset tile so
            # the PE clock is at 8/8 when the first real matmul issues.
            wz = wpool.tile([128, 512], BF16, tag="wz", name="wz")
            nc.vector.memset(wz[:], 0.0)
            pwz = bpm.tile([128, 512], F32, tag="pmm", name="warm")
            for _ in range(12):
                nc.tensor.matmul(pwz[:], wz[:, 0:128], wz[:],
                                 start=True, stop=True)

            # All transfers ordered by consumption time; the two HWDGE
            # queues (sync/scalar) share HBM BW so nothing non-critical
            # runs ahead of the phase-B critical path.
            xtc = []
            for c in range(NCH):
                xtc.append(xtp.tile([128, DB * SCH], BF16, tag=f"xtc{c}",
                                    name=f"xtc{c}"))
            xt_sb = {(d, c): xtc[c][:, d * SCH:(d + 1) * SCH]
                     for d in range(DB) for c in range(NCH)}
            # e-major consolidated weights: block (e,d) at col (e*DB+d)*128
            wkem = wpool.tile([128, DB * DB * 128], BF16, tag="wkem",
                              name="wkem")
            wqem = wpool.tile([128, DB * DB * 128], BF16, tag="wqem",
                              name="wqem")
            eb = DB * 128  # cols per e-block
            # Interleave the phase-B-critical transfers across both HWDGE
            # queues in exact consumption order (queues are FIFO, ~200-400
            # GB/s each, shared HBM):
            #   sync:   xt d0-1, xt d2-3, wk e3-5, xtc1, bv, wvm, xtc2,
            #           xtc3, wdm, bd
            #   scalar: bqk, wk e0, xt d4-5, wk e1-2, wq e0-2, wq e3-5
            # (bqk moved to scalar in the caller above... kept here: see
            # emission below)
            # Wave 1 (immediate): ONLY what the first K e-groups need.  The
            # 16 DMA engines interleave every outstanding descriptor, so
            # any extra bytes in flight delay the critical completion.
            nc.sync.dma_start(xtc[0][:, 0:2 * SCH], xt_d[0, :, 0:2 * SCH])
            nc.scalar.dma_start(wkem[:, 0:eb], wkem_d[:, 0:eb])
            nc.sync.dma_start(xtc[0][:, 2 * SCH:4 * SCH],
                              xt_d[0, :, 2 * SCH:4 * SCH])
            nc.scalar.dma_start(xtc[0][:, 4 * SCH:], xt_d[0, :, 4 * SCH:])
            nc.sync.dma_start(wkem[:, eb:3 * eb], wkem_d[:, eb:3 * eb])
            nc.scalar.dma_start(wkem[:, 3 * eb:], wkem_d[:, 3 * eb:])

            # Later waves are gated on phase-B progress via a 1-column
            # "gate copy" into the DMA target (WAW forces the DMA to wait;
            # the DMA then overwrites the corrupted column).  Tiles:
            bv_sb = consts.tile([128, D + 2], F32, tag="bv", name="bv")
            wvm = wpool.tile([128, DB * (D + 2)], BF16, tag="wvm", name="wvm")
            wdm = wdp.tile([128, DB * D], BF16, tag="wdm", name="wdm")
            bd_sb = consts.tile([128, D], F32, tag="bd", name="bd")

            def dma_wave(gate, items):
                for eng, dst, src in items:
                    nc.vector.tensor_copy(dst[:, 0:1], gate[:, 0:1])
                    eng.dma_start(dst, src)
            ident = consts.tile([128, 128], BF16, tag="ident", name="ident")
            make_identity(nc, ident[:])
            eps_sb = consts.tile([128, 1], F32, tag="eps", name="eps")
            nc.gpsimd.memset(eps_sb[:], EPS)

            def wk_sl(e, d):
                return wkem[:, (e * DB + d) * 128:(e * DB + d + 1) * 128]

            def wq_sl(e, d):
                return wqem[:, (e * DB + d) * 128:(e * DB + d + 1) * 128]

            nsb = SCH // 128  # s-blocks per chunk
            for c in range(NCH):
                for e in range(DB):
                    pk = bpm.tile([128, SCH], F32, tag="pmm", name="pmm")
                    for d in range(DB):
                        nc.tensor.matmul(pk[:], wk_sl(e, d), xt_sb[(d, c)][:],
                                         start=(d == 0), stop=(d == DB - 1))
                    kt_t = pers.tile([128, SCH], BF16, tag=f"kt{e}_{c}",
                                     name=f"kt{e}_{c}")
                    nc.scalar.activation(kt_t[:], pk[:], AF.Relu, bias=bk_sb[e])
                    kt[(e, c)] = kt_t
                    if c == 0 and e == 0:
                        dma_wave(kt_t, [
                            (nc.scalar, wqem[:, 0:3 * eb],
                             wqem_d[:, 0:3 * eb]),
                            (nc.sync, xtc[1][:], xt_d[1]),
                        ])
                    elif c == 0 and e == 1:
                        dma_wave(kt_t, [
                            (nc.sync, bv_sb[:], bv_d[:]),
                            (nc.sync, wvm[:], wv_d[:]),
                            (nc.scalar, wqem[:, 3 * eb:], wqem_d[:, 3 * eb:]),
                        ])
                for e in range(DB):
                    pq = bpm.tile([128, SCH], F32, tag="pmm", name="pmm")
                    for d in range(DB):
                        nc.tensor.matmul(pq[:], wq_sl(e, d), xt_sb[(d, c)][:],
                                         start=(d == 0), stop=(d == DB - 1))
                    qt_t = pers.tile([128, SCH], BF16, tag=f"qt{e}_{c}",
                                     name=f"qt{e}_{c}")
                    nc.scalar.activation(qt_t[:], pq[:], AF.Relu, bias=bq_sb[e])
                    qt[(e, c)] = qt_t
                    if c == 0 and e == 0:
                        dma_wave(qt_t, [
                            (nc.sync, xtc[2][:], xt_d[2]),
                            (nc.sync, xtc[3][:], xt_d[3]),
                            (nc.sync, wdm[:], wd_d[:]),
                            (nc.sync, bd_sb[:], bd_d[:]),
                        ])
                # V s-blocks (col 768 == 1.0 via bv_aug for softmax row-sums)
                for sb in range(nsb):
                    k_idx = c * nsb + sb
                    for n0, nw in ((0, 512), (512, D + 2 - 512)):
                        pv = bpm.tile([128, 512], F32, tag="pmm", name="pmm")
                        for d in range(DB):
                            nc.tensor.matmul(
                                pv[:, :nw],
                                xt_sb[(d, c)][:, sb * 128:(sb + 1) * 128],
                                wvm[:, d * (D + 2) + n0:d * (D + 2) + n0 + nw],
                                start=(d == 0), stop=(d == DB - 1))
                        vb = bvb.tile([128, 512], F32, tag="vb", name="vb")
                        nc.vector.tensor_add(vb[:, :nw], pv[:, :nw],
                                             bv_sb[:, n0:n0 + nw])
                        nc.scalar.activation(v_sb[k_idx][:, n0:n0 + nw],
                                             vb[:, :nw], AF.Relu)

        # ------- Phase C (fused): scores -> exp -> attn -> LN1 -> proj -> LN2
        with tc.tile_pool(name="cx", bufs=2) as cx, \
             tc.tile_pool(name="cxr", bufs=2) as cxr, \
             tc.tile_pool(name="cx1", bufs=1) as cx1, \
             tc.tile_pool(name="cet", bufs=2) as cet, \
             tc.tile_pool(name="cst", bufs=2, space="PSUM") as cst, \
             tc.tile_pool(name="cpa0", bufs=2, space="PSUM") as cpa0, \
             tc.tile_pool(name="cpa1", bufs=2, space="PSUM") as cpa1, \
             tc.tile_pool(name="cpp0", bufs=1, space="PSUM") as cpp0, \
             tc.tile_pool(name="cpp1", bufs=1, space="PSUM") as cpp1:
            nqb = QCH // 128  # q-blocks per chunk
            kt_per_chunk = SCH // 128

            def ln_rstd(prefix, src, negmu, split=False):
                """rstd = exp(-0.5*ln(var+eps)).  Sum-of-squares runs on
                the scalar engine (Square + accum_out); only two tiny
                stt ops stay on the DVE.  split=True squares the halves
                separately so the first can overlap the producer of the
                second (used on the kernel's last rows)."""
                sqs = cx.tile([128, D], F32, tag=f"{prefix}sqs",
                              name=f"{prefix}sqs", bufs=1)
                ssq = cx.tile([128, 1], F32, tag=f"{prefix}ssq", name=f"{prefix}ssq")
                if split:
                    ssqb = cx.tile([128, 1], F32, tag=f"{prefix}ssqb",
                                   name=f"{prefix}ssqb")
                    nc.scalar.activation(sqs[:, 0:512], src[:, 0:512],
                                         AF.Square, accum_out=ssq[:])
                    nc.scalar.activation(sqs[:, 512:D], src[:, 512:D],
                                         AF.Square, accum_out=ssqb[:])
                    nc.vector.tensor_add(ssq[:], ssq[:], ssqb[:])
                else:
                    nc.scalar.activation(sqs[:], src[:], AF.Square,
                                         accum_out=ssq[:])
                mu2e = cx.tile([128, 1], F32, tag=f"{prefix}mu2", name=f"{prefix}mu2")
                nc.vector.scalar_tensor_tensor(
                    mu2e[:], negmu[:], negmu[:], eps_sb[:],
                    op0=ALU.mult, op1=ALU.subtract)
                var = cx.tile([128, 1], F32, tag=f"{prefix}var", name=f"{prefix}var")
                nc.vector.scalar_tensor_tensor(
                    var[:], ssq[:], 1.0 / D, mu2e[:],
                    op0=ALU.mult, op1=ALU.subtract)  # = true var + eps
                lnv = cx.tile([128, 1], F32, tag=f"{prefix}lnv", name=f"{prefix}lnv")
                nc.scalar.activation(lnv[:], var[:], AF.Ln)
                rstd = cx.tile([128, 1], F32, tag=f"{prefix}rs", name=f"{prefix}rs")
                nc.scalar.activation(rstd[:], lnv[:], AF.Exp, scale=-0.5)
                return rstd

            def neg_mean(prefix, accs):
                negmu = cx.tile([128, 1], F32, tag=f"{prefix}nm", name=f"{prefix}nm")
                nc.vector.tensor_add(negmu[:], accs[0][:], accs[1][:])
                nc.vector.tensor_scalar(negmu[:], negmu[:], -1.0 / D, None,
                                        op0=ALU.mult)
                return negmu

            x_res = {}
            rb_t = {}     # centered residual bf16 per qs
            rstd1_t = {}  # LN1 rstd per qs

            def tail(c, qs):
                """transpose rb[qs] -> rb^T, dense proj on the CENTERED
                residual, y = rstd1*(pp + rb) + bd, LN2, out rows."""
                rb = rb_t[qs]
                rstd1 = rstd1_t[qs]
                last = c == NCH - 1
                if last and qs >= 2:
                    # endgame HAM keep-warm: the PE idles ~1us at a time
                    # here waiting on DVE (LN1/copy) and re-throttles to
                    # 1.2 GHz, doubling the remaining tail matmuls.  Fill
                    # the idle with dummy N=128 matmuls on the identity.
                    fill = cst.tile([128, QCH], F32, tag="pst", name="fill")
                    for _ in range(12):
                        nc.tensor.matmul(fill[:, 0:128], ident[:], ident[:],
                                         start=True, stop=True)
                # pt shares pp1's tag: bf16 768 (1.5KB) fits the slot,
                # and their uses are sequential within each qs tail
                pt = cpp1.tile([128, D], BF16, tag="pp1", name="pt")
                for d in range(DB):
                    nc.tensor.transpose(
                        pt[:, d * 128:(d + 1) * 128],
                        rb[:, d * 128:(d + 1) * 128], ident[:])
                rbt = cx1.tile([128, D], BF16, tag=f"x1t{qs}", name=f"x1t{qs}")
                # high prio: the PE's proj group waits on this copy
                with tc.high_priority(offset=300):
                    nc.vector.tensor_copy(rbt[:], pt[:])
                x1bd = cx.tile([128, D], F32, tag="x1bd", name="x1bd")
                with tc.high_priority(offset=100):
                    nc.vector.scalar_tensor_tensor(
                        x1bd[:], rb[:], rstd1[:], bd_sb[:],
                        op0=ALU.mult, op1=ALU.add)  # = X1 + bd
                pp0 = cpp0.tile([128, 512], F32, tag="pp0", name="pp0")
                for d in range(DB):
                    nc.tensor.matmul(pp0[:], rbt[:, d * 128:(d + 1) * 128],
                                     wdm[:, d * D:d * D + 512],
                                     start=(d == 0), stop=(d == DB - 1))
                pp1 = cpp1.tile([128, 256], F32, tag="pp1", name="pp1")
                for d in range(DB):
                    nc.tensor.matmul(pp1[:], rbt[:, d * 128:(d + 1) * 128],
                                     wdm[:, d * D + 512:d * D + D],
                                     start=(d == 0), stop=(d == DB - 1))
                # y = rstd1*pp + (X1 + bd);  pp = (r-mu) @ Wd so this is
                # exactly X1 + X1 @ Wd + bd
                y_t = cx.tile([128, D], F32, tag="y_t", name="y_t")
                t0 = cx.tile([128, 1], F32, tag="t0", name="t0")
                t1 = cx.tile([128, 1], F32, tag="t1", name="t1")
                nc.vector.scalar_tensor_tensor(
                    y_t[:, 0:512], pp0[:], rstd1[:], x1bd[:, 0:512],
                    op0=ALU.mult, op1=ALU.add, accum_out=t0[:])
                nc.vector.scalar_tensor_tensor(
                    y_t[:, 512:D], pp1[:], rstd1[:], x1bd[:, 512:D],
                    op0=ALU.mult, op1=ALU.add, accum_out=t1[:])
                negmu2 = neg_mean("l2", (t0, t1))
                rstd2 = ln_rstd("l2", y_t, negmu2,
                                split=(last and qs == nqb - 1))
                nmr2 = cx.tile([128, 1], F32, tag="l2nmr", name="l2nmr")
                nc.vector.tensor_mul(nmr2[:], negmu2[:], rstd2[:])
                out_t = cx.tile([128, D], BF16, tag="out_t", name="out_t")
                r0 = c * QCH + qs * 128
                if last and qs == nqb - 1:
                    # final rows: split the LN2 apply across ACT+DVE and
                    # store in two halves so the kernel's last DMA starts
                    # ~1us earlier
                    nc.scalar.activation(out_t[:, 0:384], y_t[:, 0:384],
                                         AF.Identity, bias=nmr2[:],
                                         scale=rstd2[:])
                    nc.vector.tensor_scalar(out_t[:, 384:D], y_t[:, 384:D],
                                            rstd2[:], nmr2[:],
                                            op0=ALU.mult, op1=ALU.add)
                    nc.sync.dma_start(out_d[r0:r0 + 128, 0:384],
                                      out_t[:, 0:384])
                    nc.scalar.dma_start(out_d[r0:r0 + 128, 384:D],
                                        out_t[:, 384:D])
                else:
                    nc.scalar.activation(out_t[:], y_t[:], AF.Identity,
                                         bias=nmr2[:], scale=rstd2[:])
                    nc.sync.dma_start(out_d[r0:r0 + 128, :], out_t[:])

            pending_tail = None
            for c in range(NCH):
                # residual rows for this chunk: one DMA on the sync queue
                xrc = cxr.tile([128, nqb * D], F32, tag="xrc", name="xrc")
                nc.sync.dma_start(xrc[:], xr_d[c])
                for qs in range(nqb):
                    x_res[qs] = xrc[:, qs * D:(qs + 1) * D]
                # E^T = exp(K Q^T) per k-block, stored bf16 (scores < ~72,
                # exp stays in fp32/bf16 range without max subtraction).
                # The previous chunk's last-qs tail is emitted after two
                # score groups so its LN1 latency hides under them.
                et = []
                for k in range(SB):
                    pst = cst.tile([128, QCH], F32, tag="pst", name="pst")
                    for e in range(DB):
                        nc.tensor.matmul(
                            pst[:],
                            kt[(e, k // kt_per_chunk)][
                                :, (k % kt_per_chunk) * 128:
                                   (k % kt_per_chunk + 1) * 128],
                            qt[(e, c)][:], start=(e == 0), stop=(e == DB - 1))
                    et_t = cet.tile([128, QCH], BF16, tag=f"et{k}", name=f"et{k}")
                    nc.scalar.activation(et_t[:], pst[:], AF.Exp)
                    et.append(et_t)
                    if k == 2 and pending_tail is not None:
                        tail(*pending_tail)
                        pending_tail = None
                # attn + rowsum -> normalize + residual -> center -> rb;
                # the qs tail (transpose/proj/LN2) trails one step behind.
                for qs in range(nqb):
                    pa0 = cpa0.tile([128, 512], F32, tag="pa0", name="pa0")
                    pa1 = cpa1.tile([128, D + 2 - 512], F32, tag="pa1",
                                    name="pa1")
                    for k in range(SB):
                        nc.tensor.matmul(pa0[:],
                                         et[k][:, qs * 128:(qs + 1) * 128],
                                         v_sb[k][:, 0:512],
                                         start=(k == 0), stop=(k == SB - 1))
                    for k in range(SB):
                        nc.tensor.matmul(pa1[:],
                                         et[k][:, qs * 128:(qs + 1) * 128],
                                         v_sb[k][:, 512:D + 2],
                                         start=(k == 0), stop=(k == SB - 1))
                    # read pa1 first (high prio) so its single bank frees
                    # for qs+1 as early as the DVE can get to it
                    rcp = cx.tile([128, 1], F32, tag="rcp", name="rcp")
                    r_t = cx.tile([128, D], F32, tag="r_t", name="r_t")
                    s0 = cx.tile([128, 1], F32, tag="s0", name="s0")
                    s1 = cx.tile([128, 1], F32, tag="s1", name="s1")
                    with tc.high_priority(offset=200):
                        nc.vector.reciprocal(rcp[:], pa1[:, 256:257])
                        nc.vector.scalar_tensor_tensor(
                            r_t[:, 512:D], pa1[:, 0:256], rcp[:],
                            x_res[qs][:, 512:D],
                            op0=ALU.mult, op1=ALU.add, accum_out=s1[:])
                    nc.vector.scalar_tensor_tensor(
                        r_t[:, 0:512], pa0[:], rcp[:], x_res[qs][:, 0:512],
                        op0=ALU.mult, op1=ALU.add, accum_out=s0[:])
                    negmu = neg_mean("l1", (s0, s1))
                    # centered residual in bf16: unblocks transpose+proj
                    # without waiting for var/rstd.  On the scalar engine
                    # (Identity + per-partition bias) so the transpose
                    # path doesn't queue behind DVE backlog.
                    rb = cx1.tile([128, D], BF16, tag=f"x1_{qs}",
                                  name=f"x1_{qs}")
                    nc.scalar.activation(rb[:], r_t[:], AF.Identity,
                                         bias=negmu[:])
                    rb_t[qs] = rb
                    rstd1_t[qs] = ln_rstd("l1", r_t, negmu)
                    if qs >= 1:
                        tail(c, qs - 1)
                if c == NCH - 1:
                    tail(c, nqb - 1)
                else:
                    pending_tail = (c, nqb - 1)

    _split_matmul_waits(nc)

    # Compile with the ACT table list masked to the one set that covers
    # relu/exp/ln/square/identity, so walrus can't scatter them over
    # multiple sets (set ids keep their positions; membership is masked).
    orig_tables = bacc.get_activation_tables
    import functools

    @functools.cache
    def forced_tables(arch):
        t = orig_tables(arch)
        return {k: (v if k == ACT_SET else set()) for k, v in t.items()}

    bacc.get_activation_tables = forced_tables
    try:
        nc.compile()
    finally:
        bacc.get_activation_tables = orig_tables
    return nc


_NC_CACHE = None


def _get_nc():
    global _NC_CACHE
    if _NC_CACHE is None:
        _NC_CACHE = _build()
    return _NC_CACHE


def _prep_in_maps(X, Wq, bq, Wk, bk, Wv, bv, Wd, bd):
    X = np.ascontiguousarray(X, np.float32)
    bf = lambda a: np.ascontiguousarray(np.asarray(a, np.float32)).astype(BF16NP)
    # e-major [128, (e*DB+d)*128+j] = W[d*128+p, e*128+j]
    em = lambda W: np.ascontiguousarray(
        bf(W).reshape(DB, 128, DB, 128).transpose(1, 2, 0, 3)
        .reshape(128, DB * DB * 128))
    wq_em = em(Wq)
    wk_em = em(Wk)
    wv_aug = np.zeros((D, D + 2), np.float32)
    wv_aug[:, :D] = Wv
    wv_m = np.ascontiguousarray(
        bf(wv_aug).reshape(DB, 128, D + 2).transpose(1, 0, 2)
        .reshape(128, DB * (D + 2)))
    wd_m = np.ascontiguousarray(
        bf(Wd).reshape(DB, 128, D).transpose(1, 0, 2).reshape(128, DB * D))
    bv_aug = np.zeros((1, D + 2), np.float32)
    bv_aug[0, :D] = bv
    bv_aug[0, D] = 1.0
    bv_aug = np.ascontiguousarray(np.broadcast_to(bv_aug, (128, D + 2)))
    bd_b = np.ascontiguousarray(
        np.broadcast_to(np.asarray(bd, np.float32).reshape(1, D), (128, D)))
    shared = {
        "wkem": wk_em, "wqem": wq_em, "wv": wv_m, "wd": wd_m,
        "bqk": np.ascontiguousarray(np.concatenate(
            [np.asarray(bq, np.float32).reshape(DB, 128, 1),
             np.asarray(bk, np.float32).reshape(DB, 128, 1)], axis=0)
            .transpose(1, 0, 2).reshape(128, 2 * DB)),
        "bv": bv_aug, "bd": bd_b,
    }
    maps = []
    for c in range(N_CORES):
        # xt chunk-major: [c][p, d*SCH+s] = X[c*SCH+s, d*128+p]
        xt = np.ascontiguousarray(
            bf(X[c].T).reshape(DB, 128, NCH, SCH).transpose(2, 1, 0, 3)
            .reshape(NCH, 128, DB * SCH))
        # residual rows chunk-major: [c][p, qs*D+col] = X[c*512+qs*128+p, col]
        xr = np.ascontiguousarray(
            X[c].reshape(NCH, QCH // 128, 128, D).transpose(0, 2, 1, 3)
            .reshape(NCH, 128, (QCH // 128) * D))
        maps.append(dict(shared, xt=xt, xr=xr))
    return maps


def _run(inputs, trace=False, trace_kwargs=None):
    in_maps = _prep_in_maps(
        inputs["X"], inputs["Wq"], inputs["bq"], inputs["Wk"], inputs["bk"],
        inputs["Wv"], inputs["bv"], inputs["Wd"], inputs["bd"])
    nc = _get_nc()
    res = run_bass_kernel_spmd(nc, in_maps, list(range(N_CORES)),
                               trace=trace, **(trace_kwargs or {}))
    out = np.stack([np.asarray(res.results[c]["out"]).astype(np.float32)
                    for c in range(N_CORES)])
    return out, res


def kernel(X, Wq, bq, Wk, bk, Wv, bv, Wd, bd, g1, b1, g2, b2):
    out, _ = _run(dict(X=X, Wq=Wq, bq=bq, Wk=Wk, bk=bk, Wv=Wv, bv=bv,
                       Wd=Wd, bd=bd))
    g1 = np.asarray(g1); b1 = np.asarray(b1)
    g2 = np.asarray(g2); b2 = np.asarray(b2)
    # The kernel folds the (identity) LN affines away; handle the general
    # case anyway. A non-identity g1/b1 feeds the dense layer and cannot be
    # patched after the fact -> recompute on host (never hit for this
    # problem's deterministic inputs: g=1, b=0).
    if not (np.allclose(g1, 1.0) and np.allclose(b1, 0.0)):
        return _host_reference(X, Wq, bq, Wk, bk, Wv, bv, Wd, bd, g1, b1, g2, b2)
    if not (np.allclose(g2, 1.0) and np.allclose(b2, 0.0)):
        out = out * np.asarray(g2) + np.asarray(b2)
    return out.astype(np.float32)


def _host_reference(X, Wq, bq, Wk, bk, Wv, bv, Wd, bd, g1, b1, g2, b2):
    X = np.asarray(X, np.float64)
    out = np.empty_like(X)
    for c in range(X.shape[0]):
        x = X[c]
        Q = np.maximum(x @ Wq + bq, 0)
        K = np.maximum(x @ Wk + bk, 0)
        V = np.maximum(x @ Wv + bv, 0)
        Sc = Q @ K.T
        Sc -= Sc.max(-1, keepdims=True)
        E = np.exp(Sc)
        A = (E @ V) / E.sum(-1, keepdims=True)
        X1 = x + A
        X1 = (X1 - X1.mean(-1, keepdims=True)) / np.sqrt(
            X1.var(-1, keepdims=True) + EPS) * g1 + b1
        X2 = X1 + X1 @ Wd + bd
        X2 = (X2 - X2.mean(-1, keepdims=True)) / np.sqrt(
            X2.var(-1, keepdims=True) + EPS) * g2 + b2
        out[c] = X2
    return out.astype(np.float32)
